# Initial kernel scaffold
#
"""PointNet feature-upsampling kernel for Trainium2 (8 NeuronCores).

Strategy (data-parallel over batch, 2 batches/core):
  - KNN: negd2[n,s] = 2*x.y - |x|^2 - |y|^2 - penalty  via one augmented
    matmul (contract dim 5).  w_all = 1/(d2+eps) via ACT Ln+Exp;
    top-5 selection via DVE max8 + thresholded scalar_tensor_tensor with
    fused row-sum (accum_out); row-normalize; cast bf16.
  - interp is never materialized: y0 = p1 @ W0l^T + (W @ p2) @ W0r^T
    == p1 @ W0l^T + W @ (p2 @ W0r^T)   (P2W precomputed per batch on PE).
  - Masked BN stats via PE ones-trick: lhsT=point-mask column, rhs=y0 /
    y0^2; global (sum, sumsq) all-reduced across the 8 cores (a dummy
    warm-up AllReduce at kernel start absorbs the ~60us firmware wakeup).
  - BN apply in channel-major (one ACT Relu(scale*x+bias) per tile) after
    a DMA-xbar transpose of y0; layer-1 apply in row-major with
    replicated coefficient tiles.
"""

import sys

for _p in ("/opt/trn_rl_repo",):
    if _p not in sys.path:
        sys.path.insert(0, _p)

import numpy as np
import ml_dtypes

BF = ml_dtypes.bfloat16

import concourse.bass as bass
import concourse.bacc as bacc
import concourse.mybir as mybir
import concourse.tile as tile
from concourse import bass_utils

F32 = mybir.dt.float32
BF16 = mybir.dt.bfloat16
AF = mybir.ActivationFunctionType
ALU = mybir.AluOpType

B, N, S, D = 16, 2048, 512, 384
C0 = 768          # concat channels (= W0 in), also W0 out
C2 = 384          # W1 out
NCORES = 8
BPC = B // NCORES  # batches per core
NT = N // 128      # 16 n-tiles
ST = S // 128      # 4 s-tiles
CT0 = C0 // 128    # 6 channel tiles after layer0
KNN_EPS = float(np.finfo(np.float32).eps)
BN_EPS = 1e-5
BIGNEG = -1e10

_CACHE = {}
DEBUG = False


def _build_nc():
    nc = bacc.Bacc("TRN2", target_bir_lowering=False, debug=False,
                   num_devices=NCORES)
    for v in (KNN_EPS, BN_EPS):
        ct = nc.alloc_sbuf_tensor(f"const-f32-{v}", [128, 1], F32)
        nc.gpsimd.memset(ct.ap(), v)
        nc.const_aps.aps[(F32, v)] = ct.ap()
    nc.all_engine_barrier()

    augx_d = nc.dram_tensor("augx", [BPC, 5, N], F32, kind="ExternalInput")
    augy_d = nc.dram_tensor("augy", [BPC, 5, S], F32, kind="ExternalInput")
    p1t_d = nc.dram_tensor("p1t", [BPC, D, N], BF16, kind="ExternalInput")
    p2t_d = nc.dram_tensor("p2t", [BPC, D, S], BF16, kind="ExternalInput")
    w0lt_d = nc.dram_tensor("w0lt", [D, C0], BF16, kind="ExternalInput")
    w0rt_d = nc.dram_tensor("w0rt", [D, C0], BF16, kind="ExternalInput")
    w1t_d = nc.dram_tensor("w1t", [C0, C2], BF16, kind="ExternalInput")
    pm_d = nc.dram_tensor("pm", [BPC, N, 1], BF16, kind="ExternalInput")
    g0_d = nc.dram_tensor("g0c", [C0, 1], F32, kind="ExternalInput")
    nb0_d = nc.dram_tensor("nb0c", [C0, 1], F32, kind="ExternalInput")
    g1_d = nc.dram_tensor("g1r", [1, C2], F32, kind="ExternalInput")
    b1_d = nc.dram_tensor("b1r", [1, C2], F32, kind="ExternalInput")
    invc_d = nc.dram_tensor("invc", [128, 1], F32, kind="ExternalInput")
    ident_d = nc.dram_tensor("ident", [128, 128], F32, kind="ExternalInput")
    ones_d = nc.dram_tensor("ones1", [1, 128], F32, kind="ExternalInput")
    identb_d = nc.dram_tensor("identb", [128, 128], BF16, kind="ExternalInput")
    dum_d = nc.dram_tensor("dum", [1, 8], F32, kind="ExternalInput")
    out_d = nc.dram_tensor("out", [BPC, N, C2], BF16, kind="ExternalOutput")
    if DEBUG:
        dbgw_d = nc.dram_tensor("dbgw", [128, S], BF16, kind="ExternalOutput")
        dbgy0_d = nc.dram_tensor("dbgy0", [128, C0], BF16, kind="ExternalOutput")
        dbgst_d = nc.dram_tensor("dbgst", [2, C0], F32, kind="ExternalOutput")
        dbgpre_d = nc.dram_tensor("dbgpre", [1, 1536], F32, kind="ExternalOutput")
        dbgari_d = nc.dram_tensor("dbgari", [1, 1536], F32, kind="ExternalOutput")
        dbgaro_d = nc.dram_tensor("dbgaro", [1, 1536], F32, kind="ExternalOutput")
        dbgab_d = nc.dram_tensor("dbgab", [128, 2], F32, kind="ExternalOutput")
        dbgy1_d = nc.dram_tensor("dbgy1", [128, C2], BF16, kind="ExternalOutput")
        dbgx1_d = nc.dram_tensor("dbgx1", [128, N], BF16, kind="ExternalOutput")
        dbga1_d = nc.dram_tensor("dbga1", [128, C2 * 2], BF16, kind="ExternalOutput")

    with tile.TileContext(nc) as tc:
        _emit(nc, tc, locals())
    nc.compile()
    return nc


def _emit(nc, tc, t):
    augx_d, augy_d, p1t_d, p2t_d = t["augx_d"], t["augy_d"], t["p1t_d"], t["p2t_d"]
    w0lt_d, w0rt_d, w1t_d, pm_d = t["w0lt_d"], t["w0rt_d"], t["w1t_d"], t["pm_d"]
    g0_d, nb0_d, g1_d, b1_d = t["g0_d"], t["nb0_d"], t["g1_d"], t["b1_d"]
    invc_d, ident_d, ones_d, dum_d, out_d = (
        t["invc_d"], t["ident_d"], t["ones_d"], t["dum_d"], t["out_d"])
    identb_d = t["identb_d"]

    with (
        tc.tile_pool(name="dram", bufs=1, space="DRAM") as dram,
        tc.tile_pool(name="const", bufs=1) as cst,
        tc.tile_pool(name="knn", bufs=1) as knn,
        tc.tile_pool(name="wbf", bufs=6) as wbfp,
        tc.tile_pool(name="wt", bufs=2) as wtp,
        tc.tile_pool(name="p1t", bufs=1) as p1p,
        tc.tile_pool(name="p2w", bufs=1) as p2wp,
        tc.tile_pool(name="y0", bufs=17) as y0p,
        tc.tile_pool(name="ysq", bufs=2) as ysqp,
        tc.tile_pool(name="y0t", bufs=1) as y0tp,
        tc.tile_pool(name="y1", bufs=4) as y1p,
        tc.tile_pool(name="aff", bufs=1) as affp,
        tc.tile_pool(name="outp", bufs=3) as outp,
        tc.tile_pool(name="psA", bufs=2, space="PSUM") as psA,
        tc.tile_pool(name="psB", bufs=1, space="PSUM") as psB,
        tc.tile_pool(name="psst", bufs=1, space="PSUM") as psst,
    ):
        # ---- dummy warm-up AllReduce (absorbs collective fw wakeup) ----
        dmy_i = dram.tile([1, 8], F32, name="dmyi", tag="dmyi")
        dmy_o = dram.tile([1, 8], F32, name="dmyo", tag="dmyo")
        nc.sync.dma_start(dmy_i[:], dum_d.ap())
        nc.gpsimd.collective_compute(
            "AllReduce", ALU.add, replica_groups=[list(range(NCORES))],
            ins=[dmy_i.opt()], outs=[dmy_o.opt()])
        dmy_s = cst.tile([1, 8], F32, name="dmys", tag="dmys")
        nc.sync.dma_start(dmy_s[:], dmy_o[:])

        # ---- constants ----
        w0lt = [cst.tile([128, C0], BF16, name=f"w0lt{i}", tag=f"w0lt{i}") for i in range(3)]
        w0rt = [cst.tile([128, C0], BF16, name=f"w0rt{i}", tag=f"w0rt{i}") for i in range(3)]
        w1t = [cst.tile([128, C2], BF16, name=f"w1t{i}", tag=f"w1t{i}") for i in range(CT0)]
        for i in range(3):
            nc.scalar.dma_start(w0lt[i][:], w0lt_d.ap()[i * 128:(i + 1) * 128, :])
            nc.scalar.dma_start(w0rt[i][:], w0rt_d.ap()[i * 128:(i + 1) * 128, :])
        for i in range(CT0):
            nc.scalar.dma_start(w1t[i][:], w1t_d.ap()[i * 128:(i + 1) * 128, :])
        g0c = [cst.tile([128, 1], F32, name=f"g0{i}", tag=f"g0{i}") for i in range(CT0)]
        nb0c = [cst.tile([128, 1], F32, name=f"nb0{i}", tag=f"nb0{i}") for i in range(CT0)]
        for i in range(CT0):
            nc.scalar.dma_start(g0c[i][:], g0_d.ap()[i * 128:(i + 1) * 128, :])
            nc.scalar.dma_start(nb0c[i][:], nb0_d.ap()[i * 128:(i + 1) * 128, :])
        g1r = cst.tile([1, C2], F32, name="g1r", tag="g1r")
        b1r = cst.tile([1, C2], F32, name="b1r", tag="b1r")
        nc.scalar.dma_start(g1r[:], g1_d.ap())
        nc.scalar.dma_start(b1r[:], b1_d.ap())
        invc = cst.tile([128, 1], F32, name="invc", tag="invc")
        nc.scalar.dma_start(invc[:], invc_d.ap())
        ident = cst.tile([128, 128], F32, name="ident", tag="ident")
        nc.scalar.dma_start(ident[:], ident_d.ap())
        ones1 = cst.tile([1, 128], F32, name="ones1", tag="ones1")
        nc.scalar.dma_start(ones1[:], ones_d.ap())
        identb = cst.tile([128, 128], BF16, name="identb", tag="identb")
        nc.scalar.dma_start(identb[:], identb_d.ap())
        pmt = [[cst.tile([128, 1], BF16, name=f"pm{b}_{i}", tag=f"pm{b}_{i}") for i in range(NT)]
               for b in range(BPC)]
        for b in range(BPC):
            for i in range(NT):
                nc.scalar.dma_start(
                    pmt[b][i][:], pm_d.ap()[b, i * 128:(i + 1) * 128, :])

        # stats PSUM accumulators (persist across phase)
        # layer0: stA = [sum(:512) | sq(:512)], stB = [sum(512:768)|sq(...)]
        stA = psst.tile([1, 1024], F32, name="stA", tag="stA")
        stB = psst.tile([1, 512], F32, name="stB", tag="stB")

        ar0_i = dram.tile([1, 1536], F32, name="ar0i", tag="ar0i")
        ar0_o = dram.tile([1, 1536], F32, name="ar0o", tag="ar0o")
        ar1_i = dram.tile([1, 768], F32, name="ar1i", tag="ar1i")
        ar1_o = dram.tile([1, 768], F32, name="ar1o", tag="ar1o")
        y1dr = dram.tile([BPC * N, C2], BF16, name="y1dr", tag="y1dr")

        y0sb = {}
        y0t = {}

        for bb in range(BPC):
            # =================== phase 1a: per-batch knn (both batches) ======
            p1ts = {}
            p2ws = {}
            wts = {}
            for b in [bb]:
                augx = knn.tile([5, N], F32, name="augx", tag="augx")
                augy = knn.tile([5, S], F32, name="augy", tag="augy")
                nc.sync.dma_start(augx[:], augx_d.ap()[b])
                nc.sync.dma_start(augy[:], augy_d.ap()[b])

                # ---- KNN weights, groups of 4 n-tiles (batch Ln/Exp) ----
                p2t = [p1p.tile([128, S], BF16, name=f"p2t{i}", tag=f"p2t{i}")
                       for i in range(3)]
                for i in range(3):
                    nc.sync.dma_start(p2t[i][:], p2t_d.ap()[b, i * 128:(i + 1) * 128, :])
                # ---- P2W = p2 @ W0r^T  -> [S, C0] bf16 (fills startup PE gap)
                p2w = [p2wp.tile([128, C0], BF16, name=f"p2w{s}", tag=f"p2w{s}")
                       for s in range(ST)]
                p2ws[b] = p2w
                for s in range(ST):
                    pw_ps = psA.tile([128, C0], F32, name="pw_ps", tag="big")
                    for k in range(3):
                        nc.tensor.matmul(
                            pw_ps[:, 0:512], p2t[k][:, s * 128:(s + 1) * 128],
                            w0rt[k][:, 0:512], start=(k == 0), stop=(k == 2))
                        nc.tensor.matmul(
                            pw_ps[:, 512:768], p2t[k][:, s * 128:(s + 1) * 128],
                            w0rt[k][:, 512:768], start=(k == 0), stop=(k == 2))
                    nc.vector.tensor_copy(p2w[s][:], pw_ps[:])
                wt = [wtp.tile([128, N], BF16, name=f"wt{s}", tag=f"wt{s}") for s in range(ST)]
                wts[b] = wt
                for g in range(NT // 4):
                    cps, lts, wbfs = [], [], []
                    for j in range(4):
                        nt = g * 4 + j
                        nsl = slice(nt * 128, (nt + 1) * 128)
                        d2ps = psA.tile([128, C0], F32, name="d2ps", tag="big")
                        nc.tensor.matmul(d2ps[:, 0:S], augx[:, nsl], augy[:, 0:S],
                                         start=True, stop=True)
                        # clamp rounding-negative d2 to 0 (coincident points)
                        cp = knn.tile([128, S], F32, name="cp", tag="cp", bufs=4)
                        nc.vector.tensor_scalar(cp[:], d2ps[:, 0:S], 0.0, None, ALU.min)
                        cps.append(cp)
                    for j in range(4):
                        # l = ln(eps - negd2) = ln(d2 + eps)
                        lt = knn.tile([128, S], F32, name="lt", tag="lt", bufs=4)
                        nc.scalar.activation(lt[:], cps[j][:], AF.Ln,
                                             bias=KNN_EPS, scale=-1.0)
                        lts.append(lt)
                    for j in range(4):
                        # w_all = exp(-l) = 1/(d2+eps)  (in place on lt)
                        nc.scalar.activation(lts[j][:], lts[j][:], AF.Exp, scale=-1.0)
                    for j in range(4):
                        nt = g * 4 + j
                        cp, wa = cps[j], lts[j]
                        top8 = knn.tile([128, 8], F32, name="top8", tag="top8", bufs=3)
                        nc.vector.max(top8[:], cp[:])
                        # keep the 5 smallest d2, exact f32 compare on cp
                        rsum = knn.tile([128, 1], F32, name="rsum", tag="rsum", bufs=3)
                        nc.vector.scalar_tensor_tensor(
                            wa[:], cp[:], top8[:, 4:5], wa[:],
                            ALU.is_ge, ALU.mult, accum_out=rsum[:])
                        rinv = knn.tile([128, 1], F32, name="rinv", tag="rinv", bufs=3)
                        nc.vector.reciprocal(rinv[:], rsum[:])
                        wbf = wbfp.tile([128, S], BF16, name="wbf", tag="wbf")
                        nc.vector.tensor_scalar(wbf[:], wa[:], rinv[:], None, ALU.mult)
                        if DEBUG and b == 0 and nt == 0:
                            nc.sync.dma_start(t["dbgw_d"].ap(), wbf[:])
                        wbfs.append(wbf)
                    # PE-transpose this group into wt[s][:, g*512:(g+1)*512]
                    for s in range(ST):
                        pswt = psB.tile([128, 512], BF16, name="pswt", tag="small")
                        for j in range(4):
                            nc.tensor.transpose(
                                pswt[:, j * 128:(j + 1) * 128],
                                wbfs[j][:, s * 128:(s + 1) * 128], identb[:])
                        if s % 2 == 0:
                            nc.vector.tensor_copy(
                                wt[s][:, g * 512:(g + 1) * 512], pswt[:])
                        else:
                            nc.scalar.activation(
                                wt[s][:, g * 512:(g + 1) * 512], pswt[:], AF.Copy)

            # =================== phase 1b: per-batch mm0 + stats0 ============
            for b in [bb]:
                wt = wts[b]
                p1t = [p1p.tile([128, N], BF16, name=f"p1t{i}", tag=f"p1t{i}", bufs=1)
                       for i in range(3)]
                for i in range(3):
                    nc.sync.dma_start(p1t[i][:], p1t_d.ap()[b, i * 128:(i + 1) * 128, :])
                p2w = p2ws[b]
                # ---- mm0: y0[n, :] = p1^T-blocks @ W0l^T + wT-blocks @ P2W ----
                for nt in range(NT):
                    nsl = slice(nt * 128, (nt + 1) * 128)
                    y0ps = psA.tile([128, C0], F32, name="y0ps", tag="big")
                    for k in range(3):
                        nc.tensor.matmul(y0ps[:, 0:512], p1t[k][:, nsl],
                                         w0lt[k][:, 0:512], start=(k == 0), stop=False)
                        nc.tensor.matmul(y0ps[:, 512:768], p1t[k][:, nsl],
                                         w0lt[k][:, 512:768], start=(k == 0), stop=False)
                    for s in range(ST):
                        nc.tensor.matmul(y0ps[:, 0:512], wt[s][:, nsl],
                                         p2w[s][:, 0:512], start=False, stop=(s == ST - 1))
                        nc.tensor.matmul(y0ps[:, 512:768], wt[s][:, nsl],
                                         p2w[s][:, 512:768], start=False, stop=(s == ST - 1))
                    y0 = y0p.tile([128, C0], BF16, name="y0", tag="y0")
                    nc.vector.tensor_copy(y0[:], y0ps[:])
                    y0sb[(b, nt)] = y0
                    if DEBUG and b == 0 and nt == 0:
                        nc.sync.dma_start(t["dbgy0_d"].ap(), y0[:])
                    ysq = ysqp.tile([128, C0], BF16, name="ysq", tag="ysq")
                    nc.scalar.activation(ysq[:], y0ps[:], AF.Square)
                    # stats: sum += pm^T @ y0 ; sq += pm^T @ y0^2
                    first = (b == 0 and nt == 0)
                    last = (b == BPC - 1 and nt == NT - 1)
                    nc.tensor.matmul(stA[:, 0:512], pmt[b][nt][:], y0[:, 0:512],
                                     start=first, stop=last)
                    nc.tensor.matmul(stB[:, 0:256], pmt[b][nt][:], y0[:, 512:768],
                                     start=first, stop=last)
                    nc.tensor.matmul(stA[:, 512:1024], pmt[b][nt][:], ysq[:, 0:512],
                                     start=first, stop=last)
                    nc.tensor.matmul(stB[:, 256:512], pmt[b][nt][:], ysq[:, 512:768],
                                     start=False, stop=last)

        # =================== allreduce 0 =================================
        stAs = affp.tile([1, 1024], F32, name="stAs", tag="stAs")
        stBs = affp.tile([1, 512], F32, name="stBs", tag="stBs")
        nc.vector.tensor_copy(stAs[:], stA[:])
        nc.vector.tensor_copy(stBs[:], stB[:])
        nc.sync.dma_start(ar0_i[0:1, 0:1024], stAs[:])
        nc.sync.dma_start(ar0_i[0:1, 1024:1536], stBs[:])
        if DEBUG:
            nc.sync.dma_start(t["dbgpre_d"].ap()[0:1, 0:1024], stAs[:])
            nc.sync.dma_start(t["dbgpre_d"].ap()[0:1, 1024:1536], stBs[:])
        nc.gpsimd.collective_compute(
            "AllReduce", ALU.add, replica_groups=[list(range(NCORES))],
            ins=[ar0_i.opt()], outs=[ar0_o.opt()])
        if DEBUG:
            dbgi = affp.tile([1, 1536], F32, name="dbgi", tag="stAs")
            nc.sync.dma_start(dbgi[:], ar0_i[:])
            nc.sync.dma_start(t["dbgari_d"].ap(), dbgi[:])
            dbgo = affp.tile([1, 1536], F32, name="dbgo", tag="stAs")
            nc.sync.dma_start(dbgo[:], ar0_o[:])
            nc.sync.dma_start(t["dbgaro_d"].ap(), dbgo[:])
        st0 = affp.tile([2, C0], F32, name="st0", tag="st0")
        nc.sync.dma_start(st0[0:1, 0:512], ar0_o[0:1, 0:512])
        nc.sync.dma_start(st0[1:2, 0:512], ar0_o[0:1, 512:1024])
        nc.sync.dma_start(st0[0:1, 512:768], ar0_o[0:1, 1024:1280])
        nc.sync.dma_start(st0[1:2, 512:768], ar0_o[0:1, 1280:1536])

        if DEBUG:
            nc.sync.dma_start(t["dbgst_d"].ap(), st0[:])
        # per channel-tile affine coefs: A0 = g*rsqrt(var+eps), B0 = b - mu*A0
        A0, B0 = [], []
        tpall = psst.tile([128, 12], F32, name="tpall", tag="stB")
        for c in range(CT0):
            nc.tensor.transpose(tpall[:, 2 * c:2 * c + 2],
                                st0[:, c * 128:(c + 1) * 128], ident[0:2, 0:2])
        for c in range(CT0):
            tp = tpall[:, 2 * c:2 * c + 2]
            mun = affp.tile([128, 1], F32, name="mun", tag="mun", bufs=2)
            m2 = affp.tile([128, 1], F32, name="m2", tag="m2", bufs=2)
            nc.vector.tensor_scalar(mun[:], tp[:, 0:1], invc[:], -1.0,
                                    ALU.mult, ALU.mult)
            nc.vector.tensor_scalar(m2[:], tp[:, 1:2], invc[:], None, ALU.mult)
            nvar = affp.tile([128, 1], F32, name="nvar", tag="nvar", bufs=2)
            nc.vector.scalar_tensor_tensor(nvar[:], mun[:], mun[:], m2[:],
                                           ALU.mult, ALU.subtract)
            sd = affp.tile([128, 1], F32, name="sd", tag="sd", bufs=2)
            nc.scalar.activation(sd[:], nvar[:], AF.Sqrt, bias=BN_EPS, scale=-1.0)
            rs = affp.tile([128, 1], F32, name="rs", tag="rs", bufs=2)
            nc.vector.reciprocal(rs[:], sd[:])
            a0 = affp.tile([128, 1], F32, name=f"a0_{c}", tag=f"a0_{c}")
            nc.vector.tensor_tensor(out=a0[:], in0=rs[:], in1=g0c[c][:], op=ALU.mult)
            bb = affp.tile([128, 1], F32, name=f"b0_{c}", tag=f"b0_{c}")
            nc.vector.scalar_tensor_tensor(bb[:], mun[:], a0[:], nb0c[c][:],
                                           ALU.mult, ALU.subtract)
            A0.append(a0)
            B0.append(bb)
            if DEBUG and c == 0:
                nc.sync.dma_start(t["dbgab_d"].ap()[:, 0:1], a0[:])
                nc.sync.dma_start(t["dbgab_d"].ap()[:, 1:2], bb[:])

        # =================== apply0 + mm1 + stats1 =======================
        st1A = psst.tile([1, 1024], F32, name="st1A", tag="stA")
        y1sb = {}
        for b in range(BPC):
            # y0 transpose (PE) -> y0t[ct][:, n]; b=0's transposes have
            # no dep on the allreduce and run early, overlapping mm0/allreduce
            y0t[b] = [y0tp.tile([128, N], BF16, name=f"y0t{c}", tag=f"y0t{c}")
                      for c in range(CT0)]
            for c in range(CT0):
                for g in range(NT // 4):
                    psy = psB.tile([128, 512], BF16, name="psy", tag="small")
                    for j in range(4):
                        nt = g * 4 + j
                        nc.tensor.transpose(
                            psy[:, j * 128:(j + 1) * 128],
                            y0sb[(b, nt)][:, c * 128:(c + 1) * 128], identb[:])
                    nc.vector.tensor_copy(
                        y0t[b][c][:, g * 512:(g + 1) * 512], psy[:])
            for c in range(CT0):
                for half in range(4):
                    sl = slice(half * 512, (half + 1) * 512)
                    nc.scalar.activation(y0t[b][c][:, sl], y0t[b][c][:, sl],
                                         AF.Relu, bias=B0[c][:], scale=A0[c][:])
            if DEBUG and b == 0:
                nc.sync.dma_start(t["dbgx1_d"].ap(), y0t[0][0][:])
            for nt in range(NT):
                nsl = slice(nt * 128, (nt + 1) * 128)
                y1ps = psA.tile([128, C0], F32, name="y1ps", tag="big")
                for c in range(CT0):
                    nc.tensor.matmul(y1ps[:, 0:C2], y0t[b][c][:, nsl],
                                     w1t[c][:], start=(c == 0), stop=(c == CT0 - 1))
                y1 = y1p.tile([128, C2], BF16, name="y1", tag="y1")
                nc.vector.tensor_copy(y1[:], y1ps[:, 0:C2])
                dmae = nc.sync if nt % 2 == 0 else nc.scalar
                dmae.dma_start(y1dr[b * N + nt * 128:b * N + (nt + 1) * 128, :],
                               y1[:])
                if DEBUG and b == 0 and nt == 0:
                    nc.sync.dma_start(t["dbgy1_d"].ap(), y1[:])
                ysq = ysqp.tile([128, C2], BF16, name="ysq1", tag="ysq1")
                nc.vector.scalar_tensor_tensor(ysq[:], y1[:], 1.0, y1[:],
                                               ALU.mult, ALU.mult)
                first = (b == 0 and nt == 0)
                last = (b == BPC - 1 and nt == NT - 1)
                nc.tensor.matmul(st1A[:, 0:384], pmt[b][nt][:], y1[:],
                                 start=first, stop=last)
                nc.tensor.matmul(st1A[:, 512:896], pmt[b][nt][:], ysq[:],
                                 start=first, stop=last)

        # =================== allreduce 1 =================================
        st1s = affp.tile([1, 1024], F32, name="st1s", tag="stAs")[0:1, 0:768]
        nc.vector.tensor_copy(st1s[0:1, 0:384], st1A[:, 0:384])
        nc.vector.tensor_copy(st1s[0:1, 384:768], st1A[:, 512:896])
        nc.sync.dma_start(ar1_i[0:1, 0:768], st1s[:])
        nc.gpsimd.collective_compute(
            "AllReduce", ALU.add, replica_groups=[list(range(NCORES))],
            ins=[ar1_i.opt()], outs=[ar1_o.opt()])
        st1 = affp.tile([2, C0], F32, name="st1", tag="st0")[0:1, 0:768]
        nc.sync.dma_start(st1[:], ar1_o[:])

        mun1 = affp.tile([1, C2], F32, name="mun1", tag="mun1")
        m21 = affp.tile([1, C2], F32, name="m21", tag="m21")
        nc.vector.tensor_scalar(mun1[:], st1[0:1, 0:384], invc[0:1, :], -1.0,
                                ALU.mult, ALU.mult)
        nc.vector.tensor_scalar(m21[:], st1[0:1, 384:768], invc[0:1, :], None,
                                ALU.mult)
        t1 = affp.tile([1, C2], F32, name="t1", tag="t1")
        nc.vector.tensor_tensor(out=t1[:], in0=mun1[:], in1=mun1[:], op=ALU.mult)
        nvar1 = affp.tile([1, C2], F32, name="nvar1", tag="nvar1")
        nc.vector.tensor_tensor(out=nvar1[:], in0=t1[:], in1=m21[:], op=ALU.subtract)
        sd1 = affp.tile([1, C2], F32, name="sd1", tag="sd1")
        nc.scalar.activation(sd1[:], nvar1[:], AF.Ln, bias=BN_EPS, scale=-1.0)
        rs1 = affp.tile([1, C2], F32, name="rs1", tag="rs1")
        nc.scalar.activation(rs1[:], sd1[:], AF.Exp, scale=-0.5)
        a1 = affp.tile([1, C2], F32, name="a1", tag="a1")
        nc.vector.tensor_tensor(out=a1[:], in0=rs1[:], in1=g1r[:], op=ALU.mult)
        t2 = affp.tile([1, C2], F32, name="t2", tag="t2")
        nc.vector.tensor_tensor(out=t2[:], in0=mun1[:], in1=a1[:], op=ALU.mult)
        b1v = affp.tile([1, C2], F32, name="b1v", tag="b1v")
        nc.vector.tensor_tensor(out=b1v[:], in0=t2[:], in1=b1r[:], op=ALU.add)
        # replicate to [128, C2] bf16
        a1ps = psB.tile([128, 512], F32, name="a1ps", tag="small")
        nc.tensor.matmul(a1ps[:, 0:C2], ones1[:], a1[:], start=True, stop=True)
        a1rep = affp.tile([128, C2], BF16, name="a1rep", tag="a1rep")
        nc.vector.tensor_copy(a1rep[:], a1ps[:, 0:C2])
        b1ps = psB.tile([128, 512], F32, name="b1ps", tag="small")
        nc.tensor.matmul(b1ps[:, 0:C2], ones1[:], b1v[:], start=True, stop=True)
        b1rep = affp.tile([128, C2], BF16, name="b1rep", tag="b1rep")
        nc.vector.tensor_copy(b1rep[:], b1ps[:, 0:C2])

        if DEBUG:
            nc.sync.dma_start(t["dbga1_d"].ap()[:, 0:C2], a1rep[:])
            nc.sync.dma_start(t["dbga1_d"].ap()[:, C2:2 * C2], b1rep[:])
        # =================== apply1 + store ==============================
        for b in range(BPC):
            for nt in range(NT):
                nsl = slice(nt * 128, (nt + 1) * 128)
                y1 = outp.tile([128, C2], BF16, name="y1l", tag="y1l", bufs=6)
                dmae = nc.sync if nt % 2 == 0 else nc.scalar
                dmae.dma_start(
                    y1[:], y1dr[b * N + nt * 128:b * N + (nt + 1) * 128, :])
                z = outp.tile([128, C2], BF16, name="z", tag="z", bufs=3)
                nc.vector.tensor_tensor(out=z[:], in0=y1[:], in1=a1rep[:],
                                        op=ALU.mult)
                eng = nc.vector if nt % 2 == 0 else nc.gpsimd
                eng.tensor_tensor(out=z[:], in0=z[:], in1=b1rep[:],
                                  op=ALU.add)
                of = outp.tile([128, C2], BF16, name="of", tag="of", bufs=3)
                nc.vector.tensor_scalar(of[:], z[:], 0.0, None, ALU.max)
                dmae.dma_start(out_d.ap()[b, nsl, :], of[:])


def _prep_maps(inputs):
    xyz1 = np.asarray(inputs["xyz1"], np.float32)
    xyz2 = np.asarray(inputs["xyz2"], np.float32)
    p1 = np.asarray(inputs["points1"], np.float32)
    p2 = np.asarray(inputs["points2"], np.float32)
    elens = np.asarray(inputs["embedding_lens"]).astype(np.int64)
    pmask = np.asarray(inputs["point_mask"]).astype(bool)
    W0 = np.asarray(inputs["W0"], np.float32)
    W1 = np.asarray(inputs["W1"], np.float32)
    g0 = np.asarray(inputs["g0"], np.float32)
    b0 = np.asarray(inputs["b0"], np.float32)
    g1 = np.asarray(inputs["g1"], np.float32)
    b1 = np.asarray(inputs["b1"], np.float32)

    w0lt = np.ascontiguousarray(W0[:, :D].T).astype(BF)
    w0rt = np.ascontiguousarray(W0[:, D:].T).astype(BF)
    w1t = np.ascontiguousarray(W1.T).astype(BF)
    g0c = np.ascontiguousarray(g0.reshape(C0, 1))
    nb0c = np.ascontiguousarray(-b0.reshape(C0, 1))
    g1r = np.ascontiguousarray(g1.reshape(1, C2))
    b1r = np.ascontiguousarray(b1.reshape(1, C2))
    invc = np.full((128, 1), 1.0 / float(pmask.sum()), np.float32)
    ident = np.eye(128, dtype=np.float32)
    ones1 = np.ones((1, 128), np.float32)
    dum = np.zeros((1, 8), np.float32)

    maps = []
    for ci in range(NCORES):
        sl = slice(ci * BPC, (ci + 1) * BPC)
        x1, x2 = xyz1[sl], xyz2[sl]
        augx = np.empty((BPC, 5, N), np.float32)
        augx[:, 0:3] = 2.0 * x1.transpose(0, 2, 1)
        augx[:, 3] = 1.0
        augx[:, 4] = -(x1 ** 2).sum(-1)
        augy = np.empty((BPC, 5, S), np.float32)
        augy[:, 0:3] = x2.transpose(0, 2, 1)
        pen = (np.arange(S)[None, :] >= elens[sl][:, None]).astype(np.float32) * 1e10
        augy[:, 3] = -(x2 ** 2).sum(-1) - pen
        augy[:, 4] = 1.0
        maps.append(dict(
            augx=augx,
            augy=augy,
            p1t=np.ascontiguousarray(p1[sl].transpose(0, 2, 1)).astype(BF),
            p2t=np.ascontiguousarray(p2[sl].transpose(0, 2, 1)).astype(BF),
            w0lt=w0lt, w0rt=w0rt, w1t=w1t,
            pm=pmask[sl].astype(BF).reshape(BPC, N, 1),
            g0c=g0c, nb0c=nb0c, g1r=g1r, b1r=b1r,
            invc=invc, ident=ident, ones1=ones1, dum=dum,
            identb=np.eye(128, dtype=np.float32).astype(BF),
        ))
    return maps


def kernel(**inputs) -> np.ndarray:
    if "nc" not in _CACHE:
        _CACHE["nc"] = _build_nc()
    nc = _CACHE["nc"]
    maps = _prep_maps(inputs)
    res = bass_utils.run_bass_kernel_spmd(
        nc, maps, core_ids=list(range(NCORES)),
        **_CACHE.get("run_kwargs", {}))
    if "last_res" in _CACHE or True:
        _CACHE["last_res"] = res
    out = np.concatenate([np.asarray(res.results[i]["out"], np.float32)
                          for i in range(NCORES)], axis=0)
    return out.reshape(B, N, C2)



# revision 39
# speedup vs baseline: 1.3986x; 1.3986x over previous
"""PointNet feature-upsampling kernel for Trainium2 (8 NeuronCores).

Strategy (data-parallel over batch, 2 batches/core):
  - KNN: negd2e[n,s] = 2*x.y - |x|^2 - |y|^2 - penalty - eps via one
    augmented fp32 matmul (contract dim 5; exact f32 keeps the top-5
    boundary and the 1/d2 weights baseline-accurate).
    d2pos = -min(negd2e, -eps); w_all = reciprocal_approx_fast(d2pos) on
    DVE (~18 bits, kills the ACT Ln/Exp table thrash); top-5 selection
    via DVE max8 on the exact psum negd2e + thresholded
    scalar_tensor_tensor with fused row-sum (accum_out); row-normalize on
    ACT (Copy with per-partition scale); cast bf16; PE-transpose w into
    s-major wt tiles.
  - mm0 is emitted CHANNEL-MAJOR: y0t[c, n] = W0l-as-lhsT @ p1t
    + p2w-as-lhsT @ wt  (P2W = p2 @ W0r^T precomputed per batch on PE).
    No y0 transposes; BN-0 stats via bn_stats/bn_aggr on DVE over
    mask-multiplied (Pool) full-width chunks; mask replicated to [128, N]
    via a PE ones-broadcast.  Only fast tensor_scalar/ACT ops near the
    collectives (tiny STT/TT run ~25x slower on DVE).
  - Global (sum, sumsq) all-reduced across the 8 cores in TWO halves
    (ct0..1 then ct2..5) so both collectives hide behind PE work: mm1
    runs as two passes (ct0..1 accumulated to bf16 partial in y1, then
    ct2..5 psum + in-place DVE add).  A dummy warm-up AllReduce at kernel
    start absorbs the firmware wakeup.  BN-0 apply is one ACT
    Relu(scale*x+bias) per channel-major 512-chunk, emitted after the
    psum-freeing copies so it never head-blocks the scalar queue.
  - BN-1 stats via PE ones-trick; y1 stays in SBUF in [128, 4*C2] group
    tiles; BN-1 affine computed in replicated [128, C2] form (single-
    channel row ops are slow); apply row-major on wide group tiles split
    across DVE/Pool with ACT relu; grouped output DMA (1 descriptor per
    4 n-tiles).
"""

import sys

for _p in ("/opt/trn_rl_repo",):
    if _p not in sys.path:
        sys.path.insert(0, _p)

import numpy as np
import ml_dtypes

BF = ml_dtypes.bfloat16

import concourse.bass as bass
import concourse.bacc as bacc
import concourse.mybir as mybir
import concourse.tile as tile
from concourse import bass_utils

F32 = mybir.dt.float32
BF16 = mybir.dt.bfloat16
AF = mybir.ActivationFunctionType
ALU = mybir.AluOpType

B, N, S, D = 16, 2048, 512, 384
C0 = 768          # concat channels (= W0 in), also W0 out
C2 = 384          # W1 out
NCORES = 8
BPC = B // NCORES  # batches per core
NT = N // 128      # 16 n-tiles
ST = S // 128      # 4 s-tiles
CT0 = C0 // 128    # 6 channel tiles after layer0
NCH = N // 512     # 4 512-wide n-chunks
KNN_SPLIT = False  # hi/lo double-bf16 KNN matmul (False: exact fp32, 4x PE)
AUGR = 16 if KNN_SPLIT else 5   # contract rows of the augmented KNN matmul
AUG_DT_NP = None   # set in _prep_maps
CTA = 2            # channel tiles in allreduce half A
KNN_EPS = float(np.finfo(np.float32).eps)
BN_EPS = 1e-5
CNT_LOC = float(BPC * N)   # samples per core entering bn stats (zero-filled)

_CACHE = {}


def _build_nc():
    nc = bacc.Bacc("TRN2", target_bir_lowering=False, debug=False,
                   num_devices=NCORES)
    for v in (KNN_EPS, BN_EPS):
        ct = nc.alloc_sbuf_tensor(f"const-f32-{v}", [128, 1], F32)
        nc.gpsimd.memset(ct.ap(), v)
        nc.const_aps.aps[(F32, v)] = ct.ap()
    nc.all_engine_barrier()

    AUG_DT = BF16 if KNN_SPLIT else F32
    augx_d = nc.dram_tensor("augx", [BPC, AUGR, N], AUG_DT, kind="ExternalInput")
    augy_d = nc.dram_tensor("augy", [BPC, AUGR, S], AUG_DT, kind="ExternalInput")
    p1t_d = nc.dram_tensor("p1t", [BPC, D, N], BF16, kind="ExternalInput")
    p2t_d = nc.dram_tensor("p2t", [BPC, D, S], BF16, kind="ExternalInput")
    w0lt_d = nc.dram_tensor("w0lt", [D, C0], BF16, kind="ExternalInput")
    w0rt_d = nc.dram_tensor("w0rt", [D, C0], BF16, kind="ExternalInput")
    w1t_d = nc.dram_tensor("w1t", [C0, C2], BF16, kind="ExternalInput")
    pmc_d = nc.dram_tensor("pmc", [BPC, 128, NT], BF16, kind="ExternalInput")
    pmr_d = nc.dram_tensor("pmr", [BPC, 1, N], BF16, kind="ExternalInput")
    gb0_d = nc.dram_tensor("gb0", [128, 2 * CT0], F32, kind="ExternalInput")
    g1_d = nc.dram_tensor("g1rep", [128, C2], F32, kind="ExternalInput")
    b1_d = nc.dram_tensor("b1rep", [128, C2], F32, kind="ExternalInput")
    invc_d = nc.dram_tensor("invc", [128, 1], F32, kind="ExternalInput")
    ones1_d = nc.dram_tensor("ones1", [1, 128], F32, kind="ExternalInput")
    ones1b_d = nc.dram_tensor("ones1b", [1, 128], BF16, kind="ExternalInput")
    identb_d = nc.dram_tensor("identb", [128, 128], BF16, kind="ExternalInput")
    dum_d = nc.dram_tensor("dum", [1, 8], F32, kind="ExternalInput")
    out_d = nc.dram_tensor("out", [BPC, NCH, 4, 128, C2], BF16,
                           kind="ExternalOutput")

    with tile.TileContext(nc) as tc:
        _emit(nc, tc, locals())
    nc.compile()
    return nc


def _emit(nc, tc, t):
    augx_d, augy_d, p1t_d, p2t_d = t["augx_d"], t["augy_d"], t["p1t_d"], t["p2t_d"]
    w0lt_d, w0rt_d, w1t_d = t["w0lt_d"], t["w0rt_d"], t["w1t_d"]
    pmc_d, pmr_d, gb0_d = t["pmc_d"], t["pmr_d"], t["gb0_d"]
    g1_d, b1_d, invc_d = t["g1_d"], t["b1_d"], t["invc_d"]
    ones1_d, ones1b_d, identb_d = t["ones1_d"], t["ones1b_d"], t["identb_d"]
    dum_d, out_d = t["dum_d"], t["out_d"]

    with (
        tc.tile_pool(name="dram", bufs=1, space="DRAM") as dram,
        tc.tile_pool(name="const", bufs=1) as cst,
        tc.tile_pool(name="knn", bufs=1) as knn,
        tc.tile_pool(name="wbf", bufs=4) as wbfp,
        tc.tile_pool(name="wt", bufs=1) as wtp,
        tc.tile_pool(name="p1t", bufs=1) as p1p,
        tc.tile_pool(name="p2w", bufs=1) as p2wp,
        tc.tile_pool(name="y0t", bufs=1) as y0tp,
        tc.tile_pool(name="stat", bufs=1) as stp,
        tc.tile_pool(name="y1", bufs=1) as y1p,
        tc.tile_pool(name="aff", bufs=1) as affp,
        tc.tile_pool(name="outp", bufs=3) as outp,
        tc.tile_pool(name="ps512", bufs=4, space="PSUM") as ps512,
        tc.tile_pool(name="pstp", bufs=2, space="PSUM") as pstp,
        tc.tile_pool(name="psst", bufs=1, space="PSUM") as psst,
    ):
        # ---- dummy warm-up AllReduce (absorbs collective fw wakeup) ----
        dmy_i = dram.tile([1, 8], F32, name="dmyi", tag="dmyi")
        dmy_o = dram.tile([1, 8], F32, name="dmyo", tag="dmyo")
        nc.sync.dma_start(dmy_i[:], dum_d.ap())
        nc.gpsimd.collective_compute(
            "AllReduce", ALU.add, replica_groups=[list(range(NCORES))],
            ins=[dmy_i.opt()], outs=[dmy_o.opt()])
        dmy_s = cst.tile([1, 8], F32, name="dmys", tag="dmys")
        nc.sync.dma_start(dmy_s[:], dmy_o[:])

        # ---- constants ----
        w0lt = [cst.tile([128, C0], BF16, name=f"w0lt{i}", tag=f"w0lt{i}") for i in range(3)]
        w0rt = [cst.tile([128, C0], BF16, name=f"w0rt{i}", tag=f"w0rt{i}") for i in range(3)]
        w1t = [cst.tile([128, C2], BF16, name=f"w1t{i}", tag=f"w1t{i}") for i in range(CT0)]
        identb = cst.tile([128, 128], BF16, name="identb", tag="identb")
        nc.sync.dma_start(identb[:], identb_d.ap())
        ones1b = cst.tile([1, 128], BF16, name="ones1b", tag="ones1b")
        nc.sync.dma_start(ones1b[:], ones1b_d.ap())
        for i in range(3):
            nc.sync.dma_start(w0lt[i][:], w0lt_d.ap()[i * 128:(i + 1) * 128, :])
            nc.gpsimd.dma_start(w0rt[i][:], w0rt_d.ap()[i * 128:(i + 1) * 128, :])
        for i in range(CT0):
            nc.gpsimd.dma_start(w1t[i][:], w1t_d.ap()[i * 128:(i + 1) * 128, :])
        gb0 = cst.tile([128, 2 * CT0], F32, name="gb0", tag="gb0")
        nc.scalar.dma_start(gb0[:], gb0_d.ap())
        g1rep = cst.tile([128, C2], F32, name="g1rep", tag="g1rep")
        b1repi = cst.tile([128, C2], F32, name="b1repi", tag="b1repi")
        nc.scalar.dma_start(g1rep[:], g1_d.ap())
        nc.scalar.dma_start(b1repi[:], b1_d.ap())
        invc = cst.tile([128, 1], F32, name="invc", tag="invc")
        nc.scalar.dma_start(invc[:], invc_d.ap())
        ones1 = cst.tile([1, 128], F32, name="ones1", tag="ones1")
        nc.scalar.dma_start(ones1[:], ones1_d.ap())
        pmc = [cst.tile([128, NT], BF16, name=f"pmc{b}", tag=f"pmc{b}")
               for b in range(BPC)]
        for b in range(BPC):
            nc.gpsimd.dma_start(pmc[b][:], pmc_d.ap()[b])

        arA_i = dram.tile([128, 2 * CTA], F32, name="arAi", tag="arAi")
        arA_o = dram.tile([128, 2 * CTA], F32, name="arAo", tag="arAo")
        arB_i = dram.tile([128, 2 * (CT0 - CTA)], F32, name="arBi", tag="arBi")
        arB_o = dram.tile([128, 2 * (CT0 - CTA)], F32, name="arBo", tag="arBo")
        ar1_i = dram.tile([1, 2 * C2], F32, name="ar1i", tag="ar1i")
        ar1_o = dram.tile([1, 2 * C2], F32, name="ar1o", tag="ar1o")

        # persistent across-batch state
        y0t = {}    # (b, ct) -> [128, N] bf16 channel-major
        y1big = {}  # (b, group) -> [128, 4*C2] bf16
        A0, B0 = {}, {}
        bns = [stp.tile([128, 6 * 2 * NCH], F32, name=f"bns{c}", tag=f"bns{c}")
               for c in range(CT0)]  # bn_stats 6-tuples per (batch, chunk)
        y1sb = {}

        st1A = psst.tile([1, 1024], F32, name="st1A", tag="st1A")

        def mm0_ct(b, ct, p1t, p2w, wt, mrep, fast_tail=False):
            csl = slice(ct * 128, (ct + 1) * 128)
            yct = y0tp.tile([128, N], BF16, name=f"y0t{b}_{ct}",
                            tag=f"y0t{b}_{ct}")
            y0t[(b, ct)] = yct
            pcs = [ps512.tile([128, 512], F32, name=f"pc{j}", tag="ps512")
                   for j in range(NCH)]
            for k in range(3):
                for j in range(NCH):
                    nc.tensor.matmul(
                        pcs[j][:], w0lt[k][:, csl],
                        p1t[k][:, j * 512:(j + 1) * 512],
                        start=(k == 0), stop=False)
            for s in range(ST):
                for j in range(NCH):
                    nc.tensor.matmul(
                        pcs[j][:], p2w[s][:, csl],
                        wt[s][:, j * 512:(j + 1) * 512],
                        start=False, stop=(s == ST - 1))
            for j in range(NCH):
                jsl = slice(j * 512, (j + 1) * 512)
                # fast_tail: keep the scalar queue clear (apply0 is next) and
                # shorten the stats lead into the allreduce
                if fast_tail:
                    nc.vector.tensor_copy(yct[:, jsl], pcs[j][:])
                else:
                    nc.scalar.activation(yct[:, jsl], pcs[j][:], AF.Copy)
            # masked bn stats: Pool masks full width, DVE bn_stats per chunk
            scr = stp.tile([128, N], BF16, name="scr", tag="scrv", bufs=1)
            for h in range(2):
                hsl = slice(h * (N // 2), (h + 1) * (N // 2))
                eng = nc.vector if fast_tail else nc.gpsimd
                eng.tensor_tensor(out=scr[:, hsl], in0=yct[:, hsl],
                                  in1=mrep[:, hsl], op=ALU.mult)
            for j in range(NCH):
                slot = 6 * (b * NCH + j)
                nc.vector.bn_stats(bns[ct][:, slot:slot + 6],
                                   scr[:, j * 512:(j + 1) * 512])

        def finalize_stats(cts, stacc):
            # bn_aggr + convert (mean, var) -> (sum, sumsq) per channel tile
            # (tensor_scalar/ACT only: tiny STT/TT ops run ~25x slower on DVE)
            ncts = len(cts)
            for i, c in enumerate(cts):
                mv = affp.tile([128, 2], F32, name="mv", tag="mv", bufs=2)
                nc.vector.bn_aggr(mv[:], bns[c][:])
                nc.vector.tensor_scalar(stacc[:, i:i + 1], mv[:, 0:1],
                                        CNT_LOC, None, ALU.mult)
                esq = affp.tile([128, 1], F32, name="esq", tag="esq", bufs=2)
                nc.vector.tensor_scalar(esq[:], mv[:, 0:1], mv[:, 0:1],
                                        CNT_LOC, ALU.mult, ALU.mult)
                nc.vector.tensor_scalar(stacc[:, ncts + i:ncts + i + 1],
                                        mv[:, 1:2], CNT_LOC, esq[:],
                                        ALU.mult, ALU.add)

        def affine_coefs(cts, stall, A0, B0):
            # per-ct chain of fast tensor_scalar / ACT ops (no tiny STT/TT)
            cts = list(cts)
            k = len(cts)
            for i, c in enumerate(cts):
                mun = affp.tile([128, 1], F32, name="mun", tag=f"mun{c}")
                m2 = affp.tile([128, 1], F32, name="m2", tag=f"m2{c}")
                nc.vector.tensor_scalar(mun[:], stall[:, i:i + 1], invc[:],
                                        -1.0, ALU.mult, ALU.mult)
                nc.vector.tensor_scalar(m2[:], stall[:, k + i:k + i + 1],
                                        invc[:], None, ALU.mult)
                msqn = affp.tile([128, 1], F32, name="msqn", tag=f"msqn{c}")
                nc.vector.tensor_scalar(msqn[:], mun[:], mun[:], -1.0,
                                        ALU.mult, ALU.mult)
                var = affp.tile([128, 1], F32, name="var", tag=f"var{c}")
                nc.vector.tensor_scalar(var[:], m2[:], msqn[:], None, ALU.add)
                sd = affp.tile([128, 1], F32, name="sd", tag=f"sd{c}")
                nc.scalar.activation(sd[:], var[:], AF.Sqrt, bias=BN_EPS,
                                     scale=1.0)
                rs = affp.tile([128, 1], F32, name="rs", tag=f"rs{c}")
                nc.vector.reciprocal(rs[:], sd[:])
                a0 = affp.tile([128, 1], F32, name=f"a0_{c}", tag=f"a0_{c}")
                nc.vector.tensor_scalar(a0[:], rs[:], gb0[:, c:c + 1], None,
                                        ALU.mult)
                ma = affp.tile([128, 1], F32, name="ma", tag=f"ma{c}")
                nc.vector.tensor_scalar(ma[:], mun[:], a0[:], None, ALU.mult)
                bb = affp.tile([128, 1], F32, name=f"b0_{c}", tag=f"b0_{c}")
                nc.vector.tensor_scalar(bb[:], ma[:],
                                        gb0[:, CT0 + c:CT0 + c + 1], None,
                                        ALU.add)
                A0[c] = a0
                B0[c] = bb

        for b in range(BPC):
            # =================== KNN phase ===================
            AUG_DT = BF16 if KNN_SPLIT else F32
            augx = knn.tile([AUGR, N], AUG_DT, name="augx", tag="augx")
            augy = knn.tile([AUGR, S], AUG_DT, name="augy", tag="augy")
            nc.sync.dma_start(augx[:], augx_d.ap()[b])
            nc.sync.dma_start(augy[:], augy_d.ap()[b])
            p2t = [p1p.tile([128, S], BF16, name=f"p2t{i}", tag=f"p2t{i}")
                   for i in range(3)]
            for i in range(3):
                nc.sync.dma_start(p2t[i][:], p2t_d.ap()[b, i * 128:(i + 1) * 128, :])
            p1t = [p1p.tile([128, N], BF16, name=f"p1t{i}", tag=f"p1t{i}")
                   for i in range(3)]
            for i in range(3):
                nc.sync.dma_start(p1t[i][:], p1t_d.ap()[b, i * 128:(i + 1) * 128, :])
            pmrow = cst.tile([1, N], BF16, name="pmrow", tag="pmrow")
            nc.gpsimd.dma_start(pmrow[:], pmr_d.ap()[b])

            p2w = [p2wp.tile([128, C0], BF16, name=f"p2w{s}", tag=f"p2w{s}")
                   for s in range(ST)]
            for s in range(ST):
                pwa = ps512.tile([128, 512], F32, name="pwa", tag="ps512")
                pwb = ps512.tile([128, 512], F32, name="pwb", tag="ps512")
                for k in range(3):
                    nc.tensor.matmul(pwa[:], p2t[k][:, s * 128:(s + 1) * 128],
                                     w0rt[k][:, 0:512], start=(k == 0), stop=(k == 2))
                    nc.tensor.matmul(pwb[:, 0:256], p2t[k][:, s * 128:(s + 1) * 128],
                                     w0rt[k][:, 512:768], start=(k == 0), stop=(k == 2))
                nc.scalar.activation(p2w[s][:, 0:512], pwa[:], AF.Copy)
                nc.scalar.activation(p2w[s][:, 512:768], pwb[:, 0:256], AF.Copy)

            # ---- mask broadcast to [128, N] bf16 (PE ones trick)
            mrep = knn.tile([128, N], BF16, name="mrep", tag="mrep")
            for j in range(NCH):
                mps = ps512.tile([128, 512], F32, name="mps", tag="ps512")
                nc.tensor.matmul(mps[:], ones1b[:],
                                 pmrow[0:1, j * 512:(j + 1) * 512],
                                 start=True, stop=True)
                nc.scalar.activation(mrep[:, j * 512:(j + 1) * 512], mps[:],
                                     AF.Copy)

            wt = [wtp.tile([128, N], BF16, name=f"wt{s}", tag=f"wt{s}")
                  for s in range(ST)]
            for g in range(NT // 4):
                wbfs = []
                d2ps_l = []
                for j in range(4):
                    nt = g * 4 + j
                    nsl = slice(nt * 128, (nt + 1) * 128)
                    d2ps = ps512.tile([128, 512], F32, name="d2ps", tag="ps512")
                    nc.tensor.matmul(d2ps[:], augx[:, nsl], augy[:, 0:S],
                                     start=True, stop=True)
                    d2ps_l.append(d2ps)
                for j in range(4):
                    d2ps = d2ps_l[j]
                    # d2pos = -min(negd2e, -eps) = max(d2 + eps, eps)
                    d2pos = knn.tile([128, S], F32, name="d2pos", tag="d2pos", bufs=1)
                    nc.vector.tensor_scalar(d2pos[:], d2ps[:], -KNN_EPS, -1.0,
                                            ALU.min, ALU.mult)
                    top8 = knn.tile([128, 8], F32, name="top8", tag="top8", bufs=3)
                    nc.vector.max(top8[:], d2ps[:])
                    wa = knn.tile([128, S], F32, name="wa", tag="wa", bufs=2)
                    nc.vector.reciprocal_approx_fast(wa[:], d2pos[:])
                    # keep the 5 smallest d2: exact f32 compare on psum negd2e
                    rsum = knn.tile([128, 1], F32, name="rsum", tag="rsum", bufs=3)
                    nc.vector.scalar_tensor_tensor(
                        wa[:], d2ps[:], top8[:, 4:5], wa[:],
                        ALU.is_ge, ALU.mult, accum_out=rsum[:])
                    rinv = knn.tile([128, 1], F32, name="rinv", tag="rinv", bufs=3)
                    nc.vector.reciprocal(rinv[:], rsum[:])
                    wbf = wbfp.tile([128, S], BF16, name="wbf", tag="wbf")
                    nc.scalar.activation(wbf[:], wa[:], AF.Copy, scale=rinv[:])
                    wbfs.append(wbf)
                # PE-transpose this group into wt[s][:, g*512:(g+1)*512]
                for s in range(ST):
                    pswt = pstp.tile([128, 512], BF16, name="pswt", tag="tp")
                    for j in range(4):
                        nc.tensor.transpose(
                            pswt[:, j * 128:(j + 1) * 128],
                            wbfs[j][:, s * 128:(s + 1) * 128], identb[:])
                    if s % 2 == 0:
                        nc.vector.tensor_copy(
                            wt[s][:, g * 512:(g + 1) * 512], pswt[:])
                    else:
                        nc.scalar.activation(
                            wt[s][:, g * 512:(g + 1) * 512], pswt[:], AF.Copy)

            # =================== mm0 channel-major + stats ====================
            if b < BPC - 1:
                for ct in range(CT0):
                    mm0_ct(b, ct, p1t, p2w, wt, mrep)
            else:
                for ct in range(CTA):
                    mm0_ct(b, ct, p1t, p2w, wt, mrep)
                # kick allreduce A (ct 0..CTA-1) while PE does ct CTA..5
                staccA = affp.tile([128, 2 * CTA], F32, name="staccA",
                                   tag="staccA")
                finalize_stats(range(CTA), staccA)
                nc.sync.dma_start(arA_i[:], staccA[:])
                nc.gpsimd.collective_compute(
                    "AllReduce", ALU.add, replica_groups=[list(range(NCORES))],
                    ins=[arA_i.opt()], outs=[arA_o.opt()])
                stallA = affp.tile([128, 2 * CTA], F32, name="stallA",
                                   tag="stallA")
                nc.sync.dma_start(stallA[:], arA_o[:])
                affine_coefs(range(CTA), stallA, A0, B0)
                # interleave apply0(ct<CTA) ACT chunks between the remaining
                # mm0 channel tiles so they don't head-block the scalar
                # queue; chunk j=0 first so mm1 pass A unblocks early
                def apply0_chunk(bb_, cta, j):
                    jsl = slice(j * 512, (j + 1) * 512)
                    nc.scalar.activation(y0t[(bb_, cta)][:, jsl],
                                         y0t[(bb_, cta)][:, jsl], AF.Relu,
                                         bias=B0[cta][:], scale=A0[cta][:])

                for ct in range(CTA, CT0):
                    mm0_ct(b, ct, p1t, p2w, wt, mrep,
                           fast_tail=(ct == CT0 - 1))
                for bb_ in range(BPC):
                    for j in range(NCH):
                        for cta in range(CTA):
                            apply0_chunk(bb_, cta, j)
                staccB = affp.tile([128, 2 * (CT0 - CTA)], F32, name="staccB",
                                   tag="staccB")
                finalize_stats(range(CTA, CT0), staccB)
                nc.sync.dma_start(arB_i[:], staccB[:])
                nc.gpsimd.collective_compute(
                    "AllReduce", ALU.add, replica_groups=[list(range(NCORES))],
                    ins=[arB_i.opt()], outs=[arB_o.opt()])
                stallB = affp.tile([128, 2 * (CT0 - CTA)], F32, name="stallB",
                                   tag="stallB")
                nc.sync.dma_start(stallB[:], arB_o[:])
                affine_coefs(range(CTA, CT0), stallB, A0, B0)

        # ============== mm1 pass A (hides allreduce B) ===================
        for b in range(BPC):
            for j in range(NCH):
                y1g = y1p.tile([128, 4 * C2], BF16, name=f"y1g_{b}_{j}",
                               tag=f"y1g_{b}_{j}")
                y1big[(b, j)] = y1g
                for nt in range(j * 4, j * 4 + 4):
                    nsl = slice(nt * 128, (nt + 1) * 128)
                    y1ps = ps512.tile([128, 512], F32, name="y1ps", tag="ps512")
                    for ct in range(CTA):
                        nc.tensor.matmul(y1ps[:, 0:C2], y0t[(b, ct)][:, nsl],
                                         w1t[ct][:], start=(ct == 0),
                                         stop=(ct == CTA - 1))
                    y1 = y1g[:, (nt % 4) * C2:(nt % 4 + 1) * C2]
                    y1sb[(b, nt)] = y1
                    nc.vector.tensor_copy(y1[:], y1ps[:, 0:C2])

        # ===== apply0(ct>=CTA) + mm1 pass B + in-place add + stats1 ======
        def emit_stats1(items):
            for (bb_, nt_, y1_, ysq_) in items:
                first = (bb_ == 0 and nt_ == 0)
                last = (bb_ == BPC - 1 and nt_ == NT - 1)
                nc.tensor.matmul(st1A[:, 0:384], pmc[bb_][:, nt_:nt_ + 1], y1_[:],
                                 start=first, stop=last)
                nc.tensor.matmul(st1A[:, 512:896], pmc[bb_][:, nt_:nt_ + 1],
                                 ysq_[:], start=first, stop=last)

        pending = []
        for b in range(BPC):
            for j in range(NCH):
                for ct in range(CTA, CT0):
                    jsl = slice(j * 512, (j + 1) * 512)
                    nc.scalar.activation(y0t[(b, ct)][:, jsl],
                                         y0t[(b, ct)][:, jsl], AF.Relu,
                                         bias=B0[ct][:], scale=A0[ct][:])
                group = []
                for nt in range(j * 4, j * 4 + 4):
                    nsl = slice(nt * 128, (nt + 1) * 128)
                    y1ps = ps512.tile([128, 512], F32, name="y1psB", tag="ps512")
                    for ct in range(CTA, CT0):
                        nc.tensor.matmul(y1ps[:, 0:C2], y0t[(b, ct)][:, nsl],
                                         w1t[ct][:], start=(ct == CTA),
                                         stop=(ct == CT0 - 1))
                    y1 = y1sb[(b, nt)]
                    nc.vector.tensor_tensor(out=y1[:], in0=y1ps[:, 0:C2],
                                            in1=y1[:], op=ALU.add)
                ysqg = stp.tile([128, 4 * C2], BF16, name="ysqg", tag="ysqg",
                                bufs=1)
                if b == BPC - 1 and j == NCH - 1:
                    # last group: per-nt on DVE, pipelined with the adds, so
                    # the AR1 launch isn't behind one wide Pool op
                    for nt in range(j * 4, j * 4 + 4):
                        sl = slice((nt % 4) * C2, (nt % 4 + 1) * C2)
                        nc.vector.tensor_tensor(
                            out=ysqg[:, sl], in0=y1sb[(b, nt)][:],
                            in1=y1sb[(b, nt)][:], op=ALU.mult)
                else:
                    nc.gpsimd.tensor_tensor(out=ysqg[:], in0=y1big[(b, j)][:],
                                            in1=y1big[(b, j)][:], op=ALU.mult)
                for nt in range(j * 4, j * 4 + 4):
                    group.append((b, nt, y1sb[(b, nt)],
                                  ysqg[:, (nt % 4) * C2:(nt % 4 + 1) * C2]))
                emit_stats1(pending)
                pending = group
        emit_stats1(pending)

        # =================== allreduce 1 =================================
        st1s = affp.tile([1, 2 * C2], F32, name="st1s", tag="st1s")
        nc.scalar.activation(st1s[0:1, 0:C2], st1A[:, 0:384], AF.Copy)
        nc.scalar.activation(st1s[0:1, C2:2 * C2], st1A[:, 512:896], AF.Copy)
        nc.sync.dma_start(ar1_i[:], st1s[:])
        nc.gpsimd.collective_compute(
            "AllReduce", ALU.add, replica_groups=[list(range(NCORES))],
            ins=[ar1_i.opt()], outs=[ar1_o.opt()])
        st1 = affp.tile([1, 2 * C2], F32, name="st1", tag="st1")
        nc.sync.dma_start(st1[:], ar1_o[:])

        # replicate the summed stats to [128, C2] first, then do all the
        # affine math in fast channels=128 form
        smps = pstp.tile([128, 512], F32, name="smps", tag="tp")
        nc.tensor.matmul(smps[:, 0:C2], ones1[:], st1[0:1, 0:C2],
                         start=True, stop=True)
        sqps = pstp.tile([128, 512], F32, name="sqps", tag="tp")
        nc.tensor.matmul(sqps[:, 0:C2], ones1[:], st1[0:1, C2:2 * C2],
                         start=True, stop=True)
        mun1 = affp.tile([128, C2], F32, name="mun1", tag="mun1")
        nc.vector.tensor_scalar(mun1[:], smps[:, 0:C2], invc[:], -1.0,
                                ALU.mult, ALU.mult)
        m21 = affp.tile([128, C2], F32, name="m21", tag="m21")
        nc.vector.tensor_scalar(m21[:], sqps[:, 0:C2], invc[:], None, ALU.mult)
        nvar1 = affp.tile([128, C2], F32, name="nvar1", tag="t1")
        nc.vector.tensor_tensor(out=nvar1[:], in0=mun1[:], in1=mun1[:],
                                op=ALU.mult)
        nc.vector.tensor_tensor(out=nvar1[:], in0=nvar1[:], in1=m21[:],
                                op=ALU.subtract)
        sd1 = affp.tile([128, C2], F32, name="sd1", tag="m21")
        nc.scalar.activation(sd1[:], nvar1[:], AF.Sqrt, bias=BN_EPS, scale=-1.0)
        rs1 = affp.tile([128, C2], F32, name="rs1", tag="t1")
        nc.vector.reciprocal_approx_fast(rs1[:], sd1[:])
        a1 = affp.tile([128, C2], F32, name="a1", tag="a1")
        nc.vector.tensor_tensor(out=a1[:], in0=rs1[:], in1=g1rep[:], op=ALU.mult)
        a1rep = affp.tile([128, 4 * C2], BF16, name="a1rep", tag="a1rep")
        for r in range(4):
            if r % 2 == 0:
                nc.scalar.activation(a1rep[:, r * C2:(r + 1) * C2], a1[:],
                                     AF.Copy)
            else:
                nc.vector.tensor_copy(a1rep[:, r * C2:(r + 1) * C2], a1[:])
        t2 = affp.tile([128, C2], F32, name="t2", tag="m21")
        nc.vector.tensor_tensor(out=t2[:], in0=mun1[:], in1=a1[:], op=ALU.mult)
        b1v = affp.tile([128, C2], F32, name="b1v", tag="t1")
        nc.vector.tensor_tensor(out=b1v[:], in0=t2[:], in1=b1repi[:], op=ALU.add)
        b1rep = affp.tile([128, 4 * C2], BF16, name="b1rep", tag="b1rep")
        for r in range(4):
            if r % 2 == 0:
                nc.vector.tensor_copy(b1rep[:, r * C2:(r + 1) * C2], b1v[:])
            else:
                nc.scalar.activation(b1rep[:, r * C2:(r + 1) * C2], b1v[:],
                                     AF.Copy)

        # =================== apply1 + store ==============================
        gidx = 0
        for b in range(BPC):
            for j in range(NCH):
                y1g = y1big[(b, j)]
                z = outp.tile([128, 4 * C2], BF16, name="z", tag="z", bufs=3)
                eng = nc.gpsimd if gidx % 4 == 1 else nc.vector
                eng.tensor_tensor(out=z[:], in0=y1g[:], in1=a1rep[:],
                                  op=ALU.mult)
                eng.tensor_tensor(out=z[:], in0=z[:], in1=b1rep[:],
                                  op=ALU.add)
                of = outp.tile([128, 4 * C2], BF16, name="of", tag="of", bufs=3)
                nc.scalar.activation(of[:], z[:], AF.Relu)
                dmae = nc.sync if gidx % 2 == 0 else nc.gpsimd
                dmae.dma_start(out_d.ap()[b, j].transpose([1, 0, 2]), of[:])
                gidx += 1


def _hilo(v):
    hi = v.astype(BF).astype(np.float32)
    lo = (v - hi).astype(BF).astype(np.float32)
    return hi.astype(BF), lo.astype(BF)


def _prep_maps(inputs):
    xyz1 = np.asarray(inputs["xyz1"], np.float32)
    xyz2 = np.asarray(inputs["xyz2"], np.float32)
    p1 = np.asarray(inputs["points1"], np.float32)
    p2 = np.asarray(inputs["points2"], np.float32)
    elens = np.asarray(inputs["embedding_lens"]).astype(np.int64)
    pmask = np.asarray(inputs["point_mask"]).astype(bool)
    W0 = np.asarray(inputs["W0"], np.float32)
    W1 = np.asarray(inputs["W1"], np.float32)
    g0 = np.asarray(inputs["g0"], np.float32)
    b0 = np.asarray(inputs["b0"], np.float32)
    g1 = np.asarray(inputs["g1"], np.float32)
    b1 = np.asarray(inputs["b1"], np.float32)

    w0lt = np.ascontiguousarray(W0[:, :D].T).astype(BF)
    w0rt = np.ascontiguousarray(W0[:, D:].T).astype(BF)
    w1t = np.ascontiguousarray(W1.T).astype(BF)
    # gb0: cols 0..5 = g0 per channel tile, cols 6..11 = -b0
    gb0 = np.empty((128, 2 * CT0), np.float32)
    for c in range(CT0):
        gb0[:, c] = g0[c * 128:(c + 1) * 128]
        gb0[:, CT0 + c] = b0[c * 128:(c + 1) * 128]
    g1rep = np.ascontiguousarray(np.broadcast_to(g1.reshape(1, C2), (128, C2)))
    b1rep = np.ascontiguousarray(np.broadcast_to(b1.reshape(1, C2), (128, C2)))
    invc = np.full((128, 1), 1.0 / float(pmask.sum()), np.float32)
    ones1 = np.ones((1, 128), np.float32)
    ones1b = np.ones((1, 128), np.float32).astype(BF)
    identb = np.eye(128, dtype=np.float32).astype(BF)
    dum = np.zeros((1, 8), np.float32)

    maps = []
    for ci in range(NCORES):
        sl = slice(ci * BPC, (ci + 1) * BPC)
        x1, x2 = xyz1[sl], xyz2[sl]
        # hi/lo double-bf16 split of the augmented distance matmul:
        #   negd2e = 2x.y - |x|^2 - (|y|^2 + pen + eps)
        # rows: [2xh, 2xl, 2xh, 2xl, nxh, nxl, 1, 1] x
        #       [ yh,  yh,  yl,  yl,  1,   1, nyh, nyl]
        pen = (np.arange(S)[None, :] >= elens[sl][:, None]).astype(np.float32) * 1e10
        if KNN_SPLIT:
            txh, txl = _hilo(2.0 * x1.transpose(0, 2, 1))      # [BPC, 3, N]
            yh, yl = _hilo(x2.transpose(0, 2, 1))              # [BPC, 3, S]
            nxh, nxl = _hilo(-(x1 ** 2).sum(-1))               # [BPC, N]
            nyh, nyl = _hilo(-(x2 ** 2).sum(-1) - pen - KNN_EPS)   # [BPC, S]
            one_n = np.ones((BPC, N), np.float32).astype(BF)
            one_s = np.ones((BPC, S), np.float32).astype(BF)
            augx = np.concatenate([
                txh, txl, txh, txl, nxh[:, None], nxl[:, None],
                one_n[:, None], one_n[:, None]], axis=1)
            augy = np.concatenate([
                yh, yh, yl, yl, one_s[:, None], one_s[:, None],
                nyh[:, None], nyl[:, None]], axis=1)
        else:
            augx = np.empty((BPC, 5, N), np.float32)
            augx[:, 0:3] = 2.0 * x1.transpose(0, 2, 1)
            augx[:, 3] = 1.0
            augx[:, 4] = -(x1 ** 2).sum(-1)
            augy = np.empty((BPC, 5, S), np.float32)
            augy[:, 0:3] = x2.transpose(0, 2, 1)
            augy[:, 3] = -(x2 ** 2).sum(-1) - pen - KNN_EPS
            augy[:, 4] = 1.0
        pmb = pmask[sl].astype(BF)                       # [BPC, N]
        pmc = np.ascontiguousarray(
            pmb.reshape(BPC, NT, 128).transpose(0, 2, 1))  # [BPC, 128, NT]
        pmr = pmb.reshape(BPC, 1, N)
        maps.append(dict(
            augx=np.ascontiguousarray(augx),
            augy=np.ascontiguousarray(augy),
            p1t=np.ascontiguousarray(p1[sl].transpose(0, 2, 1)).astype(BF),
            p2t=np.ascontiguousarray(p2[sl].transpose(0, 2, 1)).astype(BF),
            w0lt=w0lt, w0rt=w0rt, w1t=w1t,
            pmc=pmc, pmr=pmr,
            gb0=gb0, g1rep=g1rep, b1rep=b1rep,
            invc=invc, ones1=ones1, ones1b=ones1b, identb=identb, dum=dum,
        ))
    return maps


def kernel(**inputs) -> np.ndarray:
    if "nc" not in _CACHE:
        _CACHE["nc"] = _build_nc()
    nc = _CACHE["nc"]
    maps = _prep_maps(inputs)
    res = bass_utils.run_bass_kernel_spmd(
        nc, maps, core_ids=list(range(NCORES)),
        **_CACHE.get("run_kwargs", {}))
    _CACHE["last_res"] = res
    out = np.concatenate([np.asarray(res.results[i]["out"], np.float32)
                          .reshape(BPC, N, C2) for i in range(NCORES)], axis=0)
    return out.reshape(B, N, C2)
